# revision 1
# baseline (speedup 1.0000x reference)
"""MultiHeadDepthwiseSelfAttention Trainium2 kernel (8-core data-parallel over batch).

Math (per batch): q/k/v = depthwise-conv1d(x) (K=3, per-channel, zero pad);
heads of D=64; scores = softmax((q k^T)/sqrt(768)); out = (scores v) @ wo.T + bo.

Device layout strategy (per core, 2 batches):
- x loaded channel-major (x^T) via transposed-AP DMA; depthwise conv runs as
  per-partition fused multiply-adds (tensor_scalar + scalar_tensor_tensor).
- scores computed transposed (j on partitions) so exp feeds the attn matmul
  with no transposes; v transposed to token-major via PE transposes, stored
  with a ones column per head (augmented V) so the softmax denominator r
  falls out of the attn matmul as an extra output row.
- 1/r broadcast across partitions via tiny k=1 matmuls (PE broadcast); one
  tensor-tensor multiply per head normalizes; odd heads DMA-stacked onto
  partitions 64-127 to rebuild full feature chunks.
- output projection computed transposed (out^T = woT-chunks^T @ attn^T) so bo
  is a per-partition bias fused into the PSUM eviction; transposed-AP DMA
  stores straight to the (B, N, F) output.

All matmul operands are base-partition-0 except the even/odd conv halves for
scores (documented {0,64} auto-derivation); no explicit tile_position.
"""

import sys

sys.path.insert(0, "/opt/trn_rl_repo")

from contextlib import ExitStack

import numpy as np

import concourse.bass as bass
import concourse.tile as tile
from concourse import bacc, mybir
from concourse.masks import make_identity

F32 = mybir.dt.float32
F32R = mybir.dt.float32r

B, N, FEAT, HEAD, D, KS = 16, 512, 768, 12, 64, 3
NCORES = 8
B_LOC = B // NCORES          # batches per core
NCH = FEAT // 128            # 6 channel chunks (2 heads each)
NJB = N // 128               # 4 token blocks
MUL = mybir.AluOpType.mult
ADD = mybir.AluOpType.add

_PROG_CACHE = {}


def r32(ap):
    return ap.bitcast(F32R)


def _conv_chain(eng, out, xpad, w_sb, b_sb, c, tmp=None):
    """out[:,n] = w0*x[n-1] + w1*x[n] + w2*x[n+1] + b, channel-major chunk c.

    With final_eng/tmp set, taps 0-1 build in tmp and final_eng (DVE) writes
    `out` as float32r so it can legally feed an fp32r matmul."""
    mid = tmp if tmp is not None else out
    eng.scalar.activation(
        out=mid[:, :],
        in_=xpad[:, 0:N],
        func=mybir.ActivationFunctionType.Identity,
        bias=b_sb[:, c : c + 1],
        scale=w_sb[:, c, 0:1],
    )
    eng.vector.scalar_tensor_tensor(
        out=mid[:, :], in0=xpad[:, 1 : N + 1], scalar=w_sb[:, c, 1:2],
        in1=mid[:, :], op0=MUL, op1=ADD,
    )
    eng.vector.scalar_tensor_tensor(
        out=r32(out[:, :]) if tmp is not None else out[:, :],
        in0=xpad[:, 2 : N + 2], scalar=w_sb[:, c, 2:3],
        in1=mid[:, :], op0=MUL, op1=ADD,
    )


def build_program():
    if "nc" in _PROG_CACHE:
        return _PROG_CACHE["nc"]
    nc = bacc.Bacc("TRN2", target_bir_lowering=False)

    x_d = nc.dram_tensor("x", [B_LOC, N, FEAT], F32, kind="ExternalInput")
    wq_d = nc.dram_tensor("wq", [128, NCH, KS], F32, kind="ExternalInput")
    wk_d = nc.dram_tensor("wk", [128, NCH, KS], F32, kind="ExternalInput")
    wv_d = nc.dram_tensor("wv", [128, NCH, KS], F32, kind="ExternalInput")
    bq_d = nc.dram_tensor("bq", [128, NCH], F32, kind="ExternalInput")
    bk_d = nc.dram_tensor("bk", [128, NCH], F32, kind="ExternalInput")
    bv_d = nc.dram_tensor("bv", [128, NCH], F32, kind="ExternalInput")
    woT_d = nc.dram_tensor("woT", [FEAT, FEAT], F32, kind="ExternalInput")
    bo_d = nc.dram_tensor("bo", [128, NCH], F32, kind="ExternalInput")
    out_d = nc.dram_tensor("out", [B_LOC, N, FEAT], F32, kind="ExternalOutput")

    with tile.TileContext(nc) as tc, ExitStack() as ctx:
        consts = ctx.enter_context(tc.tile_pool(name="consts", bufs=1))
        xt_pool = ctx.enter_context(tc.tile_pool(name="xt", bufs=7))
        q_pool = ctx.enter_context(tc.tile_pool(name="qT", bufs=7))
        k_pool = ctx.enter_context(tc.tile_pool(name="kT", bufs=7))
        vt_pool = ctx.enter_context(tc.tile_pool(name="vT", bufs=7))
        va_pool = ctx.enter_context(tc.tile_pool(name="vaug", bufs=5))
        exp_pool = ctx.enter_context(tc.tile_pool(name="exp", bufs=6))
        rr_pool = ctx.enter_context(tc.tile_pool(name="rrow", bufs=3))
        bs_pool = ctx.enter_context(tc.tile_pool(name="brc_sb", bufs=3))
        at_pool = ctx.enter_context(tc.tile_pool(name="attnT", bufs=7))
        od_pool = ctx.enter_context(tc.tile_pool(name="oddtmp", bufs=2))
        ct_pool = ctx.enter_context(tc.tile_pool(name="convtmp", bufs=3))
        ot_pool = ctx.enter_context(tc.tile_pool(name="outT", bufs=3))
        ps_score = ctx.enter_context(tc.tile_pool(name="ps_score", bufs=2, space="PSUM"))
        ps_attn = ctx.enter_context(tc.tile_pool(name="ps_attn", bufs=1, space="PSUM"))
        ps_brc = ctx.enter_context(tc.tile_pool(name="ps_brc", bufs=1, space="PSUM"))
        ps_big = ctx.enter_context(tc.tile_pool(name="ps_big", bufs=1, space="PSUM"))

        # constants / weights
        ident = consts.tile([128, 128], F32)
        make_identity(nc, ident[:, :])
        ones_s = consts.tile([D + 1, 64], F32)
        nc.vector.memset(ones_s[:, :], 1.0)
        ones_m = consts.tile([D + 1, 64], F32)  # lhsT row (partition 64) for brc
        nc.vector.tensor_copy(out=r32(ones_m[D : D + 1, :]), in_=ones_s[D : D + 1, :])
        ones_c = consts.tile([128, HEAD, 1], F32)   # v_aug ones column source
        nc.vector.memset(ones_c[...], 1.0)

        wq_sb = consts.tile([128, NCH, KS], F32)
        wk_sb = consts.tile([128, NCH, KS], F32)
        wv_sb = consts.tile([128, NCH, KS], F32)
        bq_sb = consts.tile([128, NCH], F32)
        bk_sb = consts.tile([128, NCH], F32)
        bv_sb = consts.tile([128, NCH], F32)
        bo_sb = consts.tile([128, NCH], F32)
        for sb, dr in ((wq_sb, wq_d), (wk_sb, wk_d), (wv_sb, wv_d),
                       (bq_sb, bq_d), (bk_sb, bk_d), (bv_sb, bv_d),
                       (bo_sb, bo_d)):
            nc.sync.dma_start(out=sb[...], in_=dr.ap())
        woT_sb = []
        for fc in range(NCH):
            t = consts.tile([128, FEAT], F32, tag=f"woT{fc}")
            nc.sync.dma_start(out=r32(t[:, :]), in_=r32(woT_d.ap()[fc * 128 : (fc + 1) * 128, :]))
            woT_sb.append(t)

        x_ap = x_d.ap()
        out_ap = out_d.ap()

        for b in range(B_LOC):
            # ---- x^T load + depthwise conv (channel-major) ----
            qT, kT, vT = [], [], []
            for c in range(NCH):
                xt = xt_pool.tile([128, N + 2], F32)
                nc.gpsimd.memset(xt[:, 0:1], 0.0)
                nc.gpsimd.memset(xt[:, N + 1 : N + 2], 0.0)
                src = bass.AP(
                    tensor=x_ap.tensor,
                    offset=b * N * FEAT + c * 128,
                    ap=[[1, 128], [FEAT, N]],
                )
                nc.sync.dma_start(out=xt[:, 1 : N + 1], in_=src)
                qt = q_pool.tile([128, N], F32)
                kt = k_pool.tile([128, N], F32)
                vt = vt_pool.tile([128, N], F32)
                ctmp = ct_pool.tile([128, N], F32)
                _conv_chain(nc, qt, xt, wq_sb, bq_sb, c, tmp=ctmp)
                ctmp2 = ct_pool.tile([128, N], F32, tag="ctmp2")
                _conv_chain(nc, kt, xt, wk_sb, bk_sb, c, tmp=ctmp2)
                _conv_chain(nc, vt, xt, wv_sb, bv_sb, c)
                qT.append(qt)
                kT.append(kt)
                vT.append(vt)

            # ---- v to token-major (augmented with per-head ones column) ----
            v_aug = []
            for ni in range(NJB):
                tp = ps_big.tile([128, 1024], F32, tag="tp")
                for cc in range(NCH):
                    nc.tensor.transpose(
                        out=tp[:, cc * 128 : (cc + 1) * 128],
                        in_=vT[cc][:, ni * 128 : (ni + 1) * 128],
                        identity=ident[:, :],
                    )
                va = va_pool.tile([128, HEAD, D + 1], F32)
                nc.scalar.copy(
                    out=r32(va[:, :, 0:D]),
                    in_=tp[:, 0:FEAT].rearrange("p (h d) -> p h d", h=HEAD),
                )
                nc.scalar.copy(out=r32(va[:, :, D : D + 1]), in_=ones_c[...])
                v_aug.append(va)

            # ---- attention per 2-head pair ----
            attnT = []
            for pair in range(NCH):
                attn_bank = ps_attn.tile([D + 1, 1024], F32)
                brc_bank = ps_brc.tile([D, 1024], F32)
                rrow = rr_pool.tile([D + 1, 1024], F32)
                for half in (0, 1):
                    h = 2 * pair + half
                    hp = slice(64 * half, 64 * half + 64)
                    cs = slice(512 * half, 512 * half + 512)
                    exps = []
                    for jb in range(NJB):
                        sc = ps_score.tile([128, N], F32)
                        nc.tensor.matmul(
                            out=sc[:, :],
                            lhsT=r32(kT[pair][hp, jb * 128 : (jb + 1) * 128]),
                            rhs=r32(qT[pair][hp, :]),
                            start=True,
                            stop=True,
                        )
                        ex = exp_pool.tile([128, N], F32)
                        nc.scalar.activation(
                            out=r32(ex[:, :]), in_=sc[:, :],
                            func=mybir.ActivationFunctionType.Exp,
                        )
                        exps.append(ex)
                    # attn^T accumulation; ones column makes row 64 = r
                    for jc in range(NJB):
                        nc.tensor.matmul(
                            out=attn_bank[:, cs],
                            lhsT=r32(v_aug[jc][:, h, :]),
                            rhs=r32(exps[jc][:, :]),
                            start=(jc == 0),
                            stop=(jc == NJB - 1),
                        )
                # reciprocal of r rows (partition 64), then move to partition 0
                with nc.allow_low_precision(reason="f32r rounding for PE operands"):
                    nc.vector.reciprocal(
                        out=r32(rrow[D : D + 1, :]), in_=attn_bank[D : D + 1, :]
                    )
                for half in (0, 1):
                    cs = slice(512 * half, 512 * half + 512)
                    nc.tensor.matmul(
                        out=brc_bank[:, cs],
                        lhsT=r32(ones_m[D : D + 1, :]),
                        rhs=r32(rrow[D : D + 1, cs]),
                        start=True,
                        stop=True,
                    )
                brc_sb = bs_pool.tile([D, 1024], F32)
                nc.scalar.copy(out=brc_sb[:, :], in_=brc_bank[:, :])
                at = at_pool.tile([128, N], F32)
                odd = od_pool.tile([D, N], F32)
                nc.vector.tensor_mul(
                    r32(at[0:D, :]), attn_bank[0:D, 0:512], brc_sb[:, 0:512]
                )
                nc.vector.tensor_mul(
                    r32(odd[:, :]), attn_bank[0:D, 512:1024], brc_sb[:, 512:1024]
                )
                nc.sync.dma_start(out=r32(at[D:128, :]), in_=r32(odd[:, :]))
                attnT.append(at)

            # ---- output projection (transposed) + bias + store ----
            for g in range(NCH):
                pj = ps_big.tile([128, N], F32, tag="tp")
                for fc in range(NCH):
                    nc.tensor.matmul(
                        out=pj[:, :],
                        lhsT=r32(woT_sb[fc][:, g * 128 : (g + 1) * 128]),
                        rhs=r32(attnT[fc][:, :]),
                        start=(fc == 0),
                        stop=(fc == NCH - 1),
                    )
                ot = ot_pool.tile([128, N], F32)
                nc.scalar.activation(
                    out=ot[:, :], in_=pj[:, :],
                    func=mybir.ActivationFunctionType.Identity,
                    bias=bo_sb[:, g : g + 1], scale=1.0,
                )
                dst = bass.AP(
                    tensor=out_ap.tensor,
                    offset=b * N * FEAT + g * 128,
                    ap=[[1, 128], [FEAT, N]],
                )
                nc.sync.dma_start(out=dst, in_=ot[:, :])

    nc.compile()
    _PROG_CACHE["nc"] = nc
    return nc


def host_inputs(x, wq, bq, wk, bk, wv, bv, wo, bo):
    """Per-core input maps. Weight layout transforms + 1/sqrt(F) fold into q."""
    s = 1.0 / np.sqrt(np.float32(FEAT))

    def taps(w):  # (F,1,K) -> (128, NCH, K)
        return np.ascontiguousarray(
            w[:, 0, :].reshape(NCH, 128, KS).transpose(1, 0, 2)
        ).astype(np.float32)

    def cols(v):  # (F,) -> (128, NCH)
        return np.ascontiguousarray(v.reshape(NCH, 128).T).astype(np.float32)

    shared = {
        "wq": taps(wq) * s, "bq": cols(bq) * s,
        "wk": taps(wk), "bk": cols(bk),
        "wv": taps(wv), "bv": cols(bv),
        "woT": np.ascontiguousarray(wo.T).astype(np.float32),
        "bo": cols(bo),
    }
    return [
        {"x": np.ascontiguousarray(x[c * B_LOC : (c + 1) * B_LOC]).astype(np.float32),
         **shared}
        for c in range(NCORES)
    ]


def kernel(x, wq, bq, wk, bk, wv, bv, wo, bo):
    from concourse.bass_utils import run_bass_kernel_spmd

    nc = build_program()
    x = np.asarray(x)
    in_maps = host_inputs(
        x, np.asarray(wq), np.asarray(bq), np.asarray(wk), np.asarray(bk),
        np.asarray(wv), np.asarray(bv), np.asarray(wo), np.asarray(bo),
    )
    res = run_bass_kernel_spmd(nc, in_maps, list(range(NCORES)))
    out = np.concatenate([res.results[c]["out"] for c in range(NCORES)], axis=0)
    return out.astype(np.float32)



# revision 3
# speedup vs baseline: 6.7950x; 6.7950x over previous
"""MultiHeadDepthwiseSelfAttention Trainium2 kernel (8-core data-parallel over batch).

Math (per batch): q/k/v = depthwise-conv1d(x) (K=3, per-channel, zero pad);
heads of D=64; scores = softmax((q k^T)/sqrt(768)); out = (scores v) @ wo.T + bo.

v2 design (all-bf16 on device, fp32 PSUM accumulation):
- x^T (channel-major) loaded straight from DRAM via the XBAR DmaTranspose
  ucode path (bf16-only) — replaces the elementwise transposed-AP DMAs that
  dominated v1 (65536 4-byte descriptors each).
- depthwise conv as per-partition fused multiply-adds on DVE (bf16 packed
  operands hit the 4x DVE perf mode).
- scores computed transposed (j on partitions) per head; exp on Act over
  [128, 1024] PSUM pairs -> bf16 exp tiles.
- attention accumulated token-major: out[i, (h, d|1)] with a per-head
  augmented-ones column in v, so the softmax denominator r lands next to each
  head's block; normalize+evict fused in one broadcast tensor_tensor per
  head-group (rinv via DVE reciprocal on strided PSUM columns).
- attn^T rebuilt with PE transposes (bf16: 1 cycle/row) feeding a token-major
  output projection; bias folded in as a k=1 ones-row matmul; contiguous
  bf16 stores.
- emission software-pipelines batches: scores/exp of batch b interleave with
  attention/projection of batch b-1 to keep PE busy while Act drains exps.
"""

import sys

sys.path.insert(0, "/opt/trn_rl_repo")

from contextlib import ExitStack

import numpy as np

import concourse.bass as bass
import concourse.tile as tile
from concourse import bacc, mybir
from concourse.masks import make_identity

F32 = mybir.dt.float32
BF16 = mybir.dt.bfloat16

B, N, FEAT, HEAD, D, KS = 16, 512, 768, 12, 64, 3
NCORES = 8
B_LOC = B // NCORES          # batches per core
NCH = FEAT // 128            # 6 channel chunks (2 heads each)
NJB = N // 128               # 4 token blocks
MUL = mybir.AluOpType.mult
ADD = mybir.AluOpType.add
EXP = mybir.ActivationFunctionType.Exp

# at_ps head packing: heads 0-6 in bank0 (7*65=455 <= 512 fp32), 7-11 in bank1
GRPS = ((0, 0, 7), (7, 512, 5))  # (first head, col offset, nheads)

_PROG_CACHE = {}


def build_program():
    if "nc" in _PROG_CACHE:
        return _PROG_CACHE["nc"]
    nc = bacc.Bacc("TRN2", target_bir_lowering=False)

    x_d = nc.dram_tensor("x", [B_LOC, N, FEAT], BF16, kind="ExternalInput")
    # per chunk: cols 0:3 wq taps, 3:6 wk, 6:9 wv, 9 bq, 10 bk, 11 bv
    wcat_d = nc.dram_tensor("wcat", [128, NCH, 12], F32, kind="ExternalInput")
    woT_d = nc.dram_tensor("woT", [128, NCH, FEAT], BF16, kind="ExternalInput")
    bo_d = nc.dram_tensor("bo", [1, FEAT], BF16, kind="ExternalInput")
    out_d = nc.dram_tensor("out", [B_LOC, N, FEAT], BF16, kind="ExternalOutput")

    with tile.TileContext(nc) as tc, ExitStack() as ctx:
        consts = ctx.enter_context(tc.tile_pool(name="consts", bufs=1))
        xt_pool = ctx.enter_context(tc.tile_pool(name="xt", bufs=2))
        q_pool = ctx.enter_context(tc.tile_pool(name="qT", bufs=1))
        k_pool = ctx.enter_context(tc.tile_pool(name="kT", bufs=1))
        v_pool = ctx.enter_context(tc.tile_pool(name="vT", bufs=1))
        va_pool = ctx.enter_context(tc.tile_pool(name="vaug", bufs=8))
        ex_pool = ctx.enter_context(tc.tile_pool(name="exp", bufs=48))
        asb_pool = ctx.enter_context(tc.tile_pool(name="attnsb", bufs=3))
        atT_pool = ctx.enter_context(tc.tile_pool(name="attnT", bufs=2))
        ri_pool = ctx.enter_context(tc.tile_pool(name="rinv", bufs=4))
        ot_pool = ctx.enter_context(tc.tile_pool(name="outsb", bufs=3))
        ps_score = ctx.enter_context(tc.tile_pool(name="ps_score", bufs=2, space="PSUM"))
        ps_attn = ctx.enter_context(tc.tile_pool(name="ps_attn", bufs=1, space="PSUM"))
        ps_sh = ctx.enter_context(tc.tile_pool(name="ps_sh", bufs=1, space="PSUM"))

        # constants / weights
        identb = consts.tile([128, 128], BF16)
        make_identity(nc, identb[:, :])
        ones1 = consts.tile([1, 128], BF16)
        nc.vector.memset(ones1[:, :], 1.0)
        wcat_sb = consts.tile([128, NCH, 12], F32)
        woT_sb = consts.tile([128, NCH, FEAT], BF16)
        bo_sb = consts.tile([1, FEAT], BF16)
        nc.sync.dma_start(out=wcat_sb[...], in_=wcat_d.ap())
        nc.sync.dma_start(out=woT_sb[...], in_=woT_d.ap())
        nc.sync.dma_start(out=bo_sb[...], in_=bo_d.ap())

        x_ap = x_d.ap()
        out_ap = out_d.ap()

        state = {}  # per-batch tiles

        def emit_load_conv(b):
            xt = xt_pool.tile([128, NCH, N + 2], BF16)
            for c in range(NCH):
                nc.sync.dma_start_transpose(
                    out=xt[:, c, 1 : N + 1],
                    in_=x_ap[b, :, c * 128 : (c + 1) * 128],
                )
            nc.gpsimd.memset(xt[:, :, 0:1], 0.0)
            nc.gpsimd.memset(xt[:, :, N + 1 : N + 2], 0.0)

            qT = q_pool.tile([128, NCH, N], BF16)
            kT = k_pool.tile([128, NCH, N], BF16)
            vT = v_pool.tile([128, NCH, N], BF16)
            for c in range(NCH):
                for dst, wb, bcol in ((qT, 0, 9), (kT, 3, 10), (vT, 6, 11)):
                    nc.vector.tensor_scalar(
                        out=dst[:, c, :], in0=xt[:, c, 0:N],
                        scalar1=wcat_sb[:, c, wb : wb + 1],
                        scalar2=wcat_sb[:, c, bcol : bcol + 1],
                        op0=MUL, op1=ADD,
                    )
                    for t in (1, 2):
                        nc.vector.scalar_tensor_tensor(
                            out=dst[:, c, :], in0=xt[:, c, t : N + t],
                            scalar=wcat_sb[:, c, wb + t : wb + t + 1],
                            in1=dst[:, c, :], op0=MUL, op1=ADD,
                        )
            # v to token-major with per-head augmented ones column
            vas = []
            for ni in range(NJB):
                vt_ps = ps_sh.tile([128, 1024], BF16, tag="sh")
                for c in range(NCH):
                    nc.tensor.transpose(
                        out=vt_ps[:, c * 128 : (c + 1) * 128],
                        in_=vT[:, c, ni * 128 : (ni + 1) * 128],
                        identity=identb[:, :],
                    )
                va = va_pool.tile([128, HEAD, D + 1], BF16)
                nc.gpsimd.tensor_copy(
                    out=va[:, :, 0:D],
                    in_=vt_ps[:, 0:FEAT].rearrange("p (h d) -> p h d", h=HEAD),
                )
                nc.gpsimd.memset(va[:, :, D : D + 1], 1.0)
                vas.append(va)
            state[b] = {"qT": qT, "kT": kT, "va": vas, "ex": [None] * HEAD}

        def emit_scores(b, h):
            st = state[b]
            pair, half = h // 2, h % 2
            hp = slice(64 * half, 64 * half + 64)
            exs = []
            for hjb in range(2):
                sc_ps = ps_score.tile([128, 1024], F32)
                for jj in range(2):
                    jb = hjb * 2 + jj
                    nc.tensor.matmul(
                        out=sc_ps[:, jj * 512 : (jj + 1) * 512],
                        lhsT=st["kT"][hp, pair, jb * 128 : (jb + 1) * 128],
                        rhs=st["qT"][hp, pair, :],
                        start=True, stop=True,
                    )
                ex = ex_pool.tile([128, 1024], BF16, tag="ex")
                nc.scalar.activation(out=ex[:, :], in_=sc_ps[:, :], func=EXP)
                exs.append(ex)
            st["ex"][h] = exs

        def emit_attn(b, i):
            st = state[b]
            at_ps = ps_attn.tile([128, 1024], F32)
            for h in range(HEAD):
                g0, coff, _ = GRPS[0] if h < GRPS[1][0] else GRPS[1]
                col = coff + 65 * (h - g0)
                for jc in range(NJB):
                    exb = st["ex"][h][jc // 2]
                    nc.tensor.matmul(
                        out=at_ps[:, col : col + 65],
                        lhsT=exb[:, (jc % 2) * 512 + i * 128 : (jc % 2) * 512 + (i + 1) * 128],
                        rhs=st["va"][jc][:, h, :],
                        start=(jc == 0), stop=(jc == NJB - 1),
                    )
            rinv = ri_pool.tile([128, HEAD, 1], F32)
            attn_sb = asb_pool.tile([128, FEAT], BF16)
            ocol = 0
            for g0, coff, nh in GRPS:
                grp = at_ps[:, coff : coff + 65 * nh].rearrange(
                    "p (h x) -> p h x", h=nh
                )
                nc.vector.reciprocal(
                    out=rinv[:, g0 : g0 + nh, :], in_=grp[:, :, D : D + 1]
                )
                nc.vector.tensor_tensor(
                    out=attn_sb[:, ocol : ocol + nh * D].rearrange(
                        "p (h d) -> p h d", h=nh
                    ),
                    in0=grp[:, :, 0:D],
                    in1=rinv[:, g0 : g0 + nh, :].broadcast_to([128, nh, D]),
                    op=MUL,
                )
                ocol += nh * D
            st.setdefault("asb", {})[i] = attn_sb

        def emit_tail(b, i):
            """attn^T rebuild + output projection + store for i-block."""
            st = state[b]
            attn_sb = st["asb"].pop(i)
            if "atT" not in st:
                st["atT"] = atT_pool.tile([128, NCH, N], BF16, name="attnT_t")
            atT = st["atT"]
            at2_ps = ps_sh.tile([128, FEAT], BF16, tag="sh", name="at2_ps")
            for c in range(NCH):
                nc.tensor.transpose(
                    out=at2_ps[:, c * 128 : (c + 1) * 128],
                    in_=attn_sb[:, c * 128 : (c + 1) * 128],
                    identity=identb[:, :],
                )
            nc.vector.tensor_copy(
                out=atT[:, :, i * 128 : (i + 1) * 128],
                in_=at2_ps[:, 0:FEAT].rearrange("p (c x) -> p c x", c=NCH),
            )
            op_ps = ps_sh.tile([128, FEAT], F32, tag="sh", name="op_ps")
            for off, w in ((0, 512), (512, 256)):
                for c in range(NCH):
                    nc.tensor.matmul(
                        out=op_ps[:, off : off + w],
                        lhsT=atT[:, c, i * 128 : (i + 1) * 128],
                        rhs=woT_sb[:, c, off : off + w],
                        start=(c == 0), stop=False,
                    )
                nc.tensor.matmul(
                    out=op_ps[:, off : off + w],
                    lhsT=ones1[:, :],
                    rhs=bo_sb[:, off : off + w],
                    start=False, stop=True,
                )
            out_sb = ot_pool.tile([128, FEAT], BF16)
            nc.gpsimd.tensor_copy(out=out_sb[:, :], in_=op_ps[:, :])
            nc.sync.dma_start(
                out=out_ap[b, i * 128 : (i + 1) * 128, :], in_=out_sb[:, :]
            )

        # software-pipelined emission: batch b scores interleave with batch
        # b-1 attention/projection so PE fills Act's exp latency.
        for b in range(B_LOC):
            emit_load_conv(b)
            for h in range(HEAD):
                emit_scores(b, h)
                if b > 0:
                    if h == 2:
                        emit_attn(b - 1, 0)
                    elif h == 5:
                        emit_attn(b - 1, 1)
                        emit_tail(b - 1, 0)
                    elif h == 8:
                        emit_attn(b - 1, 2)
                        emit_tail(b - 1, 1)
                    elif h == 11:
                        emit_attn(b - 1, 3)
                        emit_tail(b - 1, 2)
            if b > 0:
                emit_tail(b - 1, 3)
                del state[b - 1]
        last = B_LOC - 1
        emit_attn(last, 0)
        emit_attn(last, 1)
        emit_tail(last, 0)
        emit_attn(last, 2)
        emit_tail(last, 1)
        emit_attn(last, 3)
        emit_tail(last, 2)
        emit_tail(last, 3)

    nc.compile()
    _PROG_CACHE["nc"] = nc
    return nc


def host_inputs(x, wq, bq, wk, bk, wv, bv, wo, bo):
    """Per-core input maps. Weight layout transforms + 1/sqrt(F) fold into q."""
    import ml_dtypes

    bf16 = ml_dtypes.bfloat16
    s = 1.0 / np.sqrt(np.float32(FEAT))

    def taps(w):  # (F,1,K) -> (128, NCH, K)
        return np.ascontiguousarray(
            w[:, 0, :].reshape(NCH, 128, KS).transpose(1, 0, 2)
        ).astype(np.float32)

    def cols(v):  # (F,) -> (128, NCH, 1)
        return np.ascontiguousarray(v.reshape(NCH, 128).T).astype(np.float32)[
            :, :, None
        ]

    wcat = np.concatenate(
        [taps(wq) * s, taps(wk), taps(wv), cols(bq) * s, cols(bk), cols(bv)],
        axis=2,
    ).astype(np.float32)
    woT = (
        np.ascontiguousarray(wo.T)
        .astype(np.float32)
        .reshape(NCH, 128, FEAT)
        .transpose(1, 0, 2)
        .astype(bf16)
    )
    shared = {
        "wcat": np.ascontiguousarray(wcat),
        "woT": np.ascontiguousarray(woT),
        "bo": np.asarray(bo, np.float32).reshape(1, FEAT).astype(bf16),
    }
    xb = np.asarray(x, np.float32).astype(bf16)
    return [
        {"x": np.ascontiguousarray(xb[c * B_LOC : (c + 1) * B_LOC]), **shared}
        for c in range(NCORES)
    ]


def kernel(x, wq, bq, wk, bk, wv, bv, wo, bo):
    from concourse.bass_utils import run_bass_kernel_spmd

    nc = build_program()
    in_maps = host_inputs(
        np.asarray(x), np.asarray(wq), np.asarray(bq), np.asarray(wk),
        np.asarray(bk), np.asarray(wv), np.asarray(bv), np.asarray(wo),
        np.asarray(bo),
    )
    res = run_bass_kernel_spmd(nc, in_maps, list(range(NCORES)))
    out = np.concatenate(
        [np.asarray(res.results[c]["out"]) for c in range(NCORES)], axis=0
    )
    return out.astype(np.float32)
